# revision 1
# baseline (speedup 1.0000x reference)
"""Causal single-head attention on 8 NeuronCores (Trainium2, Bass/Tile), v3.

Problem: x[16,4096,128] fp32; Wq/Wk/Wv[128,128]; y = softmax(mask(QK^T/sqrt(128))) @ V.
Sharding: data-parallel over batch, 2 batches per core, no collectives.
Measured: ~237 us HW exec (vs 394 us baseline), rel err 2.4e-3.

Design (v3):
  - S^T orientation per 512-wide q-block J: S^T strip = kT_i^T @ qT_J into
    fp32 PSUM pairs [128,1024]; one wide exp ACT per pair; PV accumulates
    O^T in PSUM; softmax sums via ones^T matmuls.
  - mixed precision: q/k fp16 everywhere. P (exp output) and V are fp8e4 for
    FULL pairs, whose many-k averaging washes out quantization noise; the 4
    diagonal k-tiles of each block stay fp16 since small-q rows have few
    softmax terms and fp8 V error would land directly in y.
  - full-pair PV and sums matmuls use fp8 DoubleRow: one matmul contracts
    both k-tiles of a pair 256-deep at 2 fp8/cycle (dim 1 of the 3D APs is
    the k-tile index, so natural [seg0|seg1] layouts work unchanged).
  - sums matmuls broadcast to all 128 PSUM rows (M=128 ones stationary) and
    accumulate with start=False onto an explicitly memset bank (a start=True
    clear is per-written-row-range); the epilogue picks row 0 via a
    selector contraction that transposes sums into per-partition layout.
  - fp16 PE transposes (1 cycle/row vs fp32's 2) for the x^T build.
  - causal masks applied on the PE: a transpose-mode matmul accumulates the
    transposed mask constant into the diagonal S^T block inside the same
    PSUM accumulation group, keeping the exp dependency chain PE-only.

Schedule (the part that bought the most):
  - emission interleaves phase-A chunks with phase-B q-blocks so the static
    per-engine instruction order lets attention start while projections for
    later chunks/batches are still in flight (PE stays dense -> HAM warm).
  - phase A lives in the wide mm PSUM slots (transposes + V tiles in one
    slot's two banks, q/k projections in another); each 4-sub-tile group is
    evacuated by ONE wide DVE cast (interleave via 2-free-dim APs); q^T/k^T
    copies moved off ACT so the scalar engine does nothing but exp.
  - sums matmuls are col-tiled: 4 consecutive P^T segments go to 4 distinct
    32-column PE array groups (tile_position=(0,32g), stationary ones[128,32])
    which execute concurrently -> ~4x less PE time than full ones^T @ P^T
    per k-tile. Partials accumulate (start=False) onto an explicitly zeroed
    PSUM bank; the epilogue contracts the 4 row-bands with a 0/1 selector.
  - per-J epilogue is split: DVE evacuations (O^T copy, sums copy) emit
    immediately; the PE part (band contraction, transposes, scaling, store)
    is deferred until after the NEXT block's pairs so it never head-of-line
    blocks the in-order PE queue.
"""
import sys

if '/opt/trn_rl_repo' not in sys.path:
    sys.path.insert(0, '/opt/trn_rl_repo')

import numpy as np

B, L, D, H = 16, 4096, 128, 128
NCORES = 8
BPC = B // NCORES          # batches per core
QB = 512                   # q block width
NQB = L // QB              # 8 q blocks
KT = 128                   # k tile width
NKT = L // KT              # 32 k tiles
CHUNK = 512                # phase-A l-chunk
NCHUNK = L // CHUNK        # 8
SCALE = float(1.0 / np.sqrt(H))
NEG = -1.0e30

_cache = {}


def _build():
    import concourse.mybir as mybir
    import concourse.tile as tile
    from concourse import bacc

    f32 = mybir.dt.float32
    f16 = mybir.dt.float16
    f8 = mybir.dt.float8e4
    DR = mybir.MatmulPerfMode.DoubleRow
    EXP = mybir.ActivationFunctionType.Exp
    CPY = mybir.ActivationFunctionType.Copy

    nc = bacc.Bacc("TRN2", target_bir_lowering=False, debug=False,
                   num_devices=NCORES)
    x_ap = nc.dram_tensor("x", [BPC, L, D], f32, kind="ExternalInput").ap()
    wq_ap = nc.dram_tensor("Wq", [D, H], f32, kind="ExternalInput").ap()
    wk_ap = nc.dram_tensor("Wk", [D, H], f32, kind="ExternalInput").ap()
    wv_ap = nc.dram_tensor("Wv", [D, H], f32, kind="ExternalInput").ap()
    id_ap = nc.dram_tensor("ident", [128, 128], f32, kind="ExternalInput").ap()
    mk_ap = nc.dram_tensor("mask", [128, 128], f32, kind="ExternalInput").ap()
    sel_ap = nc.dram_tensor("sel", [128, 3], f32, kind="ExternalInput").ap()
    y_ap = nc.dram_tensor("y", [BPC, L, H], f32, kind="ExternalOutput").ap()

    with tile.TileContext(nc) as tc:
        with (
            tc.tile_pool(name="const", bufs=1) as constp,
            tc.tile_pool(name="xchunk", bufs=6) as xchp,
            tc.tile_pool(name="xt", bufs=4) as xtp,
            tc.tile_pool(name="qkv", bufs=BPC) as qkvp,
            tc.tile_pool(name="pt", bufs=8) as ptp,
            tc.tile_pool(name="otsb", bufs=2) as otsbp,
            tc.tile_pool(name="smsb", bufs=2) as smsbp,
            tc.tile_pool(name="ysb", bufs=3) as yp,
            tc.tile_pool(name="ps_mm", bufs=2, space="PSUM") as ps_mm,
            tc.tile_pool(name="ps_ot", bufs=2, space="PSUM") as ps_ot,
            tc.tile_pool(name="ps_small", bufs=1, space="PSUM") as ps_small,
            tc.tile_pool(name="ps_sums", bufs=1, space="PSUM") as ps_sums,
        ):
            # ---- constants ----
            # Constant DMAs ride the Activation hwdge queue so they don't
            # delay the first x-chunk DMAs on the sync queue.
            ident = constp.tile([128, 128], f32, tag="ident")
            nc.scalar.dma_start(ident[:], id_ap[:])
            ident_h = constp.tile([128, 128], f16, tag="ident_h")
            nc.vector.tensor_copy(ident_h[:], ident[:])
            mask = constp.tile([128, 128], f32, tag="mask")
            nc.scalar.dma_start(mask[:], mk_ap[:])
            self_f = constp.tile([128, 3], f32, tag="self_f")
            nc.scalar.dma_start(self_f[:], sel_ap[:])
            sel_h = constp.tile([128, 3], f16, tag="sel_h")
            nc.vector.tensor_copy(sel_h[:], self_f[:])
            w_h = {}
            for name, ap in (("q", wq_ap), ("k", wk_ap), ("v", wv_ap)):
                wf = constp.tile([128, 128], f32, tag=f"w{name}f")
                nc.scalar.dma_start(wf[:], ap[:])
                wh = constp.tile([128, 128], f16, tag=f"w{name}h")
                nc.vector.tensor_copy(wh[:], wf[:])
                w_h[name] = wh
            ones_f = constp.tile([128, 256], f32, tag="ones_f")
            nc.gpsimd.memset(ones_f[:], 1.0)
            ones_h = constp.tile([128, 128], f16, tag="ones_h")
            nc.vector.tensor_copy(ones_h[:], ones_f[:, 0:128])
            ones_8 = constp.tile([128, 256], f8, tag="ones_8")
            nc.vector.tensor_copy(ones_8[:], ones_f[:])

            # ---- per-batch tensors ----
            qT = {}
            kT = {}
            Vn = {}
            Vh = {}
            xvs = {}
            yvs = {}
            for b in range(BPC):
                qT[b] = qkvp.tile([128, L], f16, tag="qT", name=f"qT{b}")
                kT[b] = qkvp.tile([128, L], f16, tag="kT", name=f"kT{b}")
                Vn[b] = qkvp.tile([128, L], f8, tag="V", name=f"V{b}")
                Vh[b] = qkvp.tile([128, L], f16, tag="Vh", name=f"Vh{b}")
                xvs[b] = x_ap[b].rearrange("(c p a) d -> c p (a d)", p=128, a=4)
                yvs[b] = y_ap[b].rearrange("(g p a) h -> g p (a h)", p=128, a=4)

            # ---- phase A chunk: transpose + projections ----
            # x chunk DMA loads 2KB contiguous per partition: partition p gets
            # rows {512c+4p+a: a=0..3}. The a-th [128,128] sub-tile transposes
            # to columns l=4p+a; one wide DVE cast with a (n,c)->4c+n dest AP
            # un-interleaves all four at once.
            # phase A split into two granules so the emission interleave
            # diffuses its PE bursts between attention pairs.
            def phase_a1(b, c):
                xch = xchp.tile([128, 512], f32, tag="xch")
                nc.sync.dma_start(xch[:], xvs[b][c])
                # pre-cast to fp16 (xt is fp16 downstream anyway): fp16
                # transposes stream at 1 cycle/row vs fp32's 2
                xh = xtp.tile([128, 512], f16, tag="xh")
                nc.vector.tensor_copy(xh[:], xch[:])
                tp = ps_mm.tile([128, 512], f16, tag="mm")
                for n in range(4):
                    nc.tensor.transpose(
                        tp[:, 128 * n:128 * (n + 1)],
                        xh[:, 128 * n:128 * (n + 1)], ident_h[:])
                xt = xtp.tile([128, CHUNK], f16, tag="xt")
                nc.vector.tensor_copy(
                    xt[:].rearrange("p (c n) -> p n c", n=4),
                    tp[:].rearrange("p (n c) -> p n c", c=128))
                return xt

            def phase_a2(b, c, xt):
                # q^T, k^T chunks: [h, CHUNK] into halves of one mm slot;
                # evacuated on ACT (idle during the b0 region; DVE is not)
                pp = ps_mm.tile([128, 1024], f32, tag="mm")
                nc.tensor.matmul(pp[:, 0:512], w_h["q"][:], xt[:],
                                 start=True, stop=True)
                nc.tensor.matmul(pp[:, 512:1024], w_h["k"][:], xt[:],
                                 start=True, stop=True)
                nc.scalar.activation(
                    qT[b][:, CHUNK * c:CHUNK * (c + 1)], pp[:, 0:512], CPY)
                nc.vector.tensor_copy(
                    kT[b][:, CHUNK * c:CHUNK * (c + 1)], pp[:, 512:1024])
                # V tiles: [l,h] per 128-l sub-tile, reusing pp's first bank
                # after the q^T copy drains it
                for n in range(4):
                    nc.tensor.matmul(pp[:, 128 * n:128 * (n + 1)],
                                     xt[:, 128 * n:128 * (n + 1)],
                                     w_h["v"][:], start=True, stop=True)
                nc.vector.tensor_copy(
                    Vh[b][:, CHUNK * c:CHUNK * (c + 1)], pp[:, 0:512])
                nc.vector.tensor_copy(
                    Vn[b][:, CHUNK * c:CHUNK * (c + 1)],
                    Vh[b][:, CHUNK * c:CHUNK * (c + 1)])

            # ---- phase B block: pairs + sums, deferred PE epilogue ----
            def phase_b(b, J, feed=None):
                nkt = 4 * J + 4
                last_i = nkt - 1
                ot = ps_ot.tile([128, QB], f32, tag="ot")
                sm = ps_sums.tile([128, QB], f32, tag="sums")
                # Zero the sums bank explicitly: col-tiled matmuls below
                # accumulate with start=False onto it (a start=True clear
                # is per-written-row-range, so stale has_written rows from
                # the previous J would otherwise carry over).
                nc.vector.memset(sm[:], 0.0)

                pairs = []
                for g in range(2 * J):
                    pairs.append((2 * g, 2 * g + 1))
                pairs.append((4 * J, 4 * J + 1))
                pairs.append((4 * J + 2, 4 * J + 3))

                # sums: full pairs contract both k-tiles in one DoubleRow
                # matmul (256-deep, 2 fp8/cycle); diag pairs keep per-segment
                # fp16 matmuls. All sums write a full [128, N] broadcast (every
                # row accumulates the same totals), so the epilogue selector
                # just picks row 0 - no band bookkeeping.
                nseg_total = 2 * J + 4
                state = {"seg": 0}
                pending = []

                def flush_sums():
                    for kind, src, qoff in pending:
                        si = state["seg"]
                        last = si == nseg_total - 1
                        if kind == "dr":
                            nc.tensor.matmul(
                                sm[:, 0:QB],
                                ones_8[:, 0:256].rearrange(
                                    "p (o m) -> p o m", o=2),
                                src.rearrange("p (o q) -> p o q", o=2),
                                start=False, stop=last,
                                skip_group_check=True, perf_mode=DR)
                        else:
                            nc.tensor.matmul(
                                sm[:, qoff:],
                                ones_h[:, 0:128], src,
                                start=False, stop=last,
                                skip_group_check=True)
                        state["seg"] = si + 1
                    pending.clear()

                npv = 0
                npv_total = 2 * J + 4  # 1 per full (DoubleRow) pair, 2 per diag pair
                for pair in pairs:
                    stw = ps_mm.tile([128, 2 * QB], f32, tag="mm")
                    entries = []
                    cur = 0
                    for i in pair:
                        qoff = max(0, 128 * (i - 4 * J))
                        N = QB - qoff
                        diag_i = i >= 4 * J
                        nc.tensor.matmul(
                            stw[:, cur:cur + N],
                            kT[b][:, KT * i:KT * (i + 1)],
                            qT[b][:, QB * J + qoff:QB * (J + 1)],
                            start=True, stop=not diag_i,
                            skip_group_check=True)
                        if diag_i:
                            # causal mask added on the PE: accumulate
                            # maskT^T (= strict-lower -1e30) onto the
                            # diagonal block; keeps the exp dependency
                            # chain PE-only (no DVE hop).
                            nc.tensor.matmul(
                                stw[:, cur:cur + 128],
                                mask[:], ident[:],
                                is_transpose=True,
                                start=False, stop=True,
                                skip_group_check=True)
                        entries.append((i, qoff, cur, N))
                        cur += N
                    # full pairs run fp8 (P and V) - quantization noise
                    # averages out over their many k. Diag pairs run fp16:
                    # they dominate small-q rows where softmax has few terms
                    # and fp8 V error would show up directly in y.
                    diag = pair[0] >= 4 * J
                    ptw = ptp.tile([128, 2 * QB], f16 if diag else f8,
                                   tag="pt" if diag else "pt8")
                    nc.scalar.activation(ptw[:, :cur], stw[:, :cur], EXP,
                                         scale=SCALE)
                    if not diag:
                        # one DoubleRow matmul contracts both k-tiles
                        # (256-deep) at 2 fp8/cycle. dim 1 of both operands
                        # is the k-tile index, so the natural [seg0|seg1]
                        # layouts work as-is.
                        i0 = pair[0]
                        nc.tensor.matmul(
                            ot[:, 0:QB],
                            Vn[b][:, KT * i0:KT * (i0 + 2)].rearrange(
                                "p (o h) -> p o h", o=2),
                            ptw[:, 0:2 * QB].rearrange(
                                "p (o q) -> p o q", o=2),
                            start=(npv == 0), stop=(npv == npv_total - 1),
                            skip_group_check=True, perf_mode=DR)
                        npv += 1
                    else:
                        for i, qoff, off, N in entries:
                            nc.tensor.matmul(ot[:, qoff:],
                                             Vh[b][:, KT * i:KT * (i + 1)],
                                             ptw[:, off:off + N],
                                             start=(npv == 0),
                                             stop=(npv == npv_total - 1),
                                             skip_group_check=True)
                            npv += 1
                    if diag:
                        for i, qoff, off, N in entries:
                            pending.append(("seg", ptw[:, off:off + N], qoff))
                    else:
                        pending.append(("dr", ptw[:, 0:2 * QB], 0))
                    if len(pending) >= 4:
                        flush_sums()
                    if feed is not None and state["seg"] % 4 == 0:
                        feed()
                flush_sums()

                # early epilogue (DVE only): evacuate ot and sm promptly
                otsb = otsbp.tile([128, QB], f32, tag="otsb")
                nc.vector.tensor_copy(
                    otsb[:].rearrange("p (a c) -> p a c", c=128),
                    ot[:].rearrange("p (c a) -> p a c", a=4))
                smr = smsbp.tile([128, QB], f16, tag="smsb")
                nc.vector.tensor_copy(smr[:], sm[:])

                def late():
                    # sums bands -> per-partition layout stp[p,a] = sums[4p+a]
                    sel = sel_h[:, 1:2]
                    stp = ps_small.tile([128, 512], f32, tag="small")
                    for a in range(4):
                        nc.tensor.matmul(stp[:, a:a + 1],
                                         smr[:, a:QB:4], sel,
                                         start=(a == 0), stop=(a == 3),
                                         skip_group_check=True)
                    rcp = smsbp.tile([128, 4], f32, tag="rcp")
                    nc.vector.reciprocal(rcp[:], stp[:, 0:4])
                    # O^T -> O (sub-tile a holds q=512J+4p+a), normalize,
                    # one 256KB store per q-block
                    ys = yp.tile([128, QB], f32, tag="y")
                    op = ps_small.tile([128, 512], f32, tag="small")
                    for a in range(4):
                        nc.tensor.transpose(op[:, 128 * a:128 * (a + 1)],
                                            otsb[:, 128 * a:128 * (a + 1)],
                                            ident[:])
                        nc.vector.tensor_scalar_mul(
                            ys[:, 128 * a:128 * (a + 1)],
                            op[:, 128 * a:128 * (a + 1)], rcp[:, a:a + 1])
                    nc.sync.dma_start(yvs[b][J], ys[:])

                return late

            # ---- interleaved emission schedule ----
            # Phase-A granules drip-feed between attention pairs; before each
            # q-block the granules it depends on are forced.
            gran = []
            for b in range(BPC):
                for c in range(NCHUNK):
                    gran.append(("a1", b, c))
                    gran.append(("a2", b, c))
            gstate = {"idx": 0}
            saved = {}

            def emit_gran():
                kind, b, c = gran[gstate["idx"]]
                if kind == "a1":
                    saved[(b, c)] = phase_a1(b, c)
                else:
                    xt = saved.pop((b, c))
                    phase_a2(b, c, xt)
                gstate["idx"] += 1

            def feed():
                if gstate["idx"] < len(gran):
                    emit_gran()

            # Block order: b1's small (ramp-heavy) q-blocks interleave with
            # b0's big (dense) ones so pipeline ramps overlap dense work.
            border = [(0, J) for J in range(4)]
            for J in range(4):
                border += [(0, 4 + J), (1, J)]
            border += [(1, J) for J in range(4, NQB)]

            pending_late = None
            for b, J in border:
                while gstate["idx"] <= (b * NCHUNK + J) * 2 + 1:
                    emit_gran()
                late = phase_b(b, J, feed=feed)
                if pending_late is not None:
                    pending_late()
                pending_late = late
            if pending_late is not None:
                pending_late()
    nc.compile()
    return nc


def _host_consts():
    ident = np.eye(128, dtype=np.float32)
    kk = np.arange(128)[:, None]
    qq = np.arange(128)[None, :]
    # transposed: the kernel adds this via a PE transpose-matmul (out = M^T)
    mask = np.where(qq >= kk, 0.0, NEG).astype(np.float32).T.copy()
    sel = np.zeros((128, 3), dtype=np.float32)
    sel[0::32, 0] = 1.0   # 4 bands (J>=2)
    sel[0, 1] = 1.0       # 1 band (J=0)
    sel[0, 2] = 1.0       # 2 bands (J=1)
    sel[32, 2] = 1.0
    return ident, mask, sel


def kernel(x, Wq, Wk, Wv):
    from concourse import bass_utils

    if "nc" not in _cache:
        _cache["nc"] = _build()
    nc = _cache["nc"]

    x = np.ascontiguousarray(x, dtype=np.float32)
    ident, mask, sel = _host_consts()
    in_maps = []
    for c in range(NCORES):
        in_maps.append({
            "x": x[BPC * c:BPC * (c + 1)],
            "Wq": np.ascontiguousarray(Wq, dtype=np.float32),
            "Wk": np.ascontiguousarray(Wk, dtype=np.float32),
            "Wv": np.ascontiguousarray(Wv, dtype=np.float32),
            "ident": ident,
            "mask": mask,
            "sel": sel,
        })
    res = bass_utils.run_bass_kernel_spmd(nc, in_maps,
                                          core_ids=list(range(NCORES)))
    _cache["last_results"] = res
    y = np.concatenate([res.results[c]["y"] for c in range(NCORES)], axis=0)
    return y

